# revision 1
# baseline (speedup 1.0000x reference)
"""GNN message passing (copy_u + segment_sum) on 8 Trainium2 cores.

Strategy (edge/data parallel, per the sharding hint):
  - Host: sort edges by dst; core c owns dst range [c*N/8, (c+1)*N/8).
  - Host: pack src_emb rows as [bf16(hi) | bf16(src-hi)] pairs (256B/row, exact
    to ~1e-5 rel) and gather per-edge message rows per core ("src_emb ...
    gathered per partition"), laid out partition-major so device DMAs are
    contiguous 32KB runs per partition.
  - Device (per core): stream message tiles; for each bin of <=128 dst rows /
    S*128 edge slots, build a one-hot [edge x dst-slot] matrix on DVE
    (dstloc == iota), then segment-sum via PE matmuls accumulating hi+lo into
    one PSUM bank; copy PSUM->SBUF, batch-store bins per group.
  - Host: scatter-add the [B*128, 64] bin blocks back to the full output.
"""
import sys
sys.path.insert(0, "/opt/trn_rl_repo")
import numpy as np
import ml_dtypes

import concourse.bass as bass
import concourse.bacc as bacc
import concourse.mybir as mybir
import concourse.tile as tile
from concourse.bass_utils import run_bass_kernel_spmd

NCORES = 8
S = 9                # subtiles (of 128 edge slots) per bin
CAP = S * 128        # edge slots per bin
PAD_LOC = 200.0      # dstloc sentinel -> one-hot row all zeros
BF16 = ml_dtypes.bfloat16

_kernel_cache = {}


def _build_kernel(B):
    """Device program: uniform over cores; B bins of S subtiles each."""
    bf16 = mybir.dt.bfloat16
    f32 = mybir.dt.float32
    nc = bacc.Bacc("TRN2", target_bir_lowering=False, debug=False,
                   num_devices=NCORES)
    msg = nc.declare_dram_parameter("msg", [128, B * CAP], bf16, isOutput=False)
    dstloc = nc.declare_dram_parameter("dstloc", [128, B * S], bf16, isOutput=False)
    iota = nc.declare_dram_parameter("iota", [128, CAP], bf16, isOutput=False)
    outp = nc.declare_dram_parameter("outp", [128, B * 64], f32, isOutput=True)

    G = 14  # bins per DMA group
    n_groups = (B + G - 1) // G

    with tile.TileContext(nc) as tc:
        with tc.tile_pool(name="const", bufs=1) as cpool, \
             tc.tile_pool(name="msgs", bufs=3) as mpool, \
             tc.tile_pool(name="oh", bufs=4) as ohpool, \
             tc.tile_pool(name="acc", bufs=8, space="PSUM") as ppool, \
             tc.tile_pool(name="ost", bufs=3) as opool:
            iota_t = cpool.tile([128, CAP], bf16)
            nc.sync.dma_start(out=iota_t[:], in_=iota[:])
            dstloc_t = cpool.tile([128, B * S], bf16)
            nc.sync.dma_start(out=dstloc_t[:], in_=dstloc[:])
            iota3d = iota_t[:].rearrange("p (s f) -> p s f", s=S)

            for g in range(n_groups):
                g0 = g * G
                gs = min(G, B - g0)
                mt = mpool.tile([128, gs * CAP], bf16, tag="mt")
                nc.sync.dma_start(out=mt[:], in_=msg[:, g0 * CAP:(g0 + gs) * CAP])
                ot = opool.tile([128, gs * 64], f32, tag="ot")
                for lb in range(gs):
                    b = g0 + lb
                    oh = ohpool.tile([128, S, 128], bf16)
                    nc.vector.tensor_tensor(
                        out=oh[:],
                        in0=dstloc_t[:, b * S:(b + 1) * S].to_broadcast([128, S, 128]),
                        in1=iota3d,
                        op=mybir.AluOpType.is_equal,
                    )
                    ps = ppool.tile([128, 64], f32)
                    for s in range(S):
                        base = lb * CAP + s * 128
                        nc.tensor.matmul(ps[:], oh[:, s, :], mt[:, base:base + 64],
                                         start=(s == 0), stop=False)
                        nc.tensor.matmul(ps[:], oh[:, s, :], mt[:, base + 64:base + 128],
                                         start=False, stop=(s == S - 1))
                    nc.vector.tensor_copy(out=ot[:, lb * 64:(lb + 1) * 64], in_=ps[:])
                nc.sync.dma_start(out=outp[:, g0 * 64:(g0 + gs) * 64], in_=ot[:])
    nc.compile()
    return nc


def _pack_core(d_local, s_local, n_dst_local):
    """Greedy bins: <=128 distinct dst rows and <=CAP edges per bin.
    Returns (srcs [B,CAP] int64, locs [B,CAP] uint8->float, rows [B,128] int64
    with n_dst_local as trash)."""
    n = len(d_local)
    bins = []
    if n:
        firsts = np.flatnonzero(np.concatenate(([True], d_local[1:] != d_local[:-1])))
        nf = len(firsts)
        start = 0
        while start < n:
            j0 = np.searchsorted(firsts, start, side="right") - 1
            lim = firsts[j0 + 128] if j0 + 128 < nf else n
            end = min(start + CAP, lim)
            bins.append((start, end))
            start = end
    B = len(bins)
    srcs = np.zeros((B, CAP), dtype=np.int64)
    locs = np.full((B, CAP), PAD_LOC, dtype=np.float32)
    rows = np.full((B, 128), n_dst_local, dtype=np.int64)
    for i, (st, en) in enumerate(bins):
        m = en - st
        u, inv = np.unique(d_local[st:en], return_inverse=True)
        srcs[i, :m] = s_local[st:en]
        locs[i, :m] = inv.astype(np.float32)
        rows[i, :len(u)] = u
    return srcs, locs, rows


def kernel(src_emb, edge_src, edge_dst, num_dst):
    src_emb = np.asarray(src_emb, dtype=np.float32)
    edge_src = np.asarray(edge_src).astype(np.int64)
    edge_dst = np.asarray(edge_dst).astype(np.int64)
    n_dst = int(num_dst)
    n_src, d = src_emb.shape
    assert d == 64

    # hi/lo bf16 split: hi + lo == src exactly to ~2^-17 relative
    hi = src_emb.astype(BF16)
    lo = (src_emb - hi.astype(np.float32)).astype(BF16)
    packed = np.concatenate([hi, lo], axis=1)  # [n_src, 128] bf16

    # dst-sorted edge partition across cores
    order = np.argsort(edge_dst, kind="stable")
    ds = edge_dst[order]
    ss = edge_src[order]
    per = (n_dst + NCORES - 1) // NCORES
    cuts = np.searchsorted(ds, np.arange(1, NCORES) * per)
    d_parts = np.split(ds, cuts)
    s_parts = np.split(ss, cuts)

    cores = []
    for c in range(NCORES):
        dl = d_parts[c] - c * per
        nl = min(per, n_dst - c * per)
        cores.append(_pack_core(dl, s_parts[c], nl))
    B = max(cr[0].shape[0] for cr in cores)

    iota_np = np.tile(np.arange(128, dtype=np.float32), S)[None, :].repeat(128, 0).astype(BF16)

    in_maps = []
    rows_g = []
    for c, (srcs, locs, rows) in enumerate(cores):
        b0 = srcs.shape[0]
        if b0 < B:
            srcs = np.concatenate([srcs, np.zeros((B - b0, CAP), np.int64)])
            locs = np.concatenate([locs, np.full((B - b0, CAP), PAD_LOC, np.float32)])
            nl = min(per, n_dst - c * per)
            rows = np.concatenate([rows, np.full((B - b0, 128), nl, np.int64)])
        # [128, B*S*128] partition-major messages
        msg_np = packed[srcs.reshape(B * S, 128).T].reshape(128, -1)
        dstloc_np = locs.reshape(B * S, 128).T.astype(BF16).copy()
        in_maps.append({"msg": msg_np, "dstloc": dstloc_np, "iota": iota_np})
        nl = min(per, n_dst - c * per)
        # local trash sentinel nl -> dedicated global trash slot n_dst + c
        rows_g.append(np.where(rows == nl, n_dst + c, rows + c * per))

    if B not in _kernel_cache:
        _kernel_cache[B] = _build_kernel(B)
    nc = _kernel_cache[B]
    res = run_bass_kernel_spmd(nc, in_maps, core_ids=list(range(NCORES)))

    full = np.zeros((n_dst + NCORES, 64), dtype=np.float32)
    for c in range(NCORES):
        blocks = res.results[c]["outp"].reshape(128, B, 64).transpose(1, 0, 2)
        np.add.at(full, rows_g[c].ravel(), blocks.reshape(B * 128, 64))
    return full[:n_dst]


if __name__ == "__main__":
    rng = np.random.default_rng(1)
    ns, nd, e = 1000, 1000, 5000
    semb = rng.standard_normal((ns, 64), dtype=np.float32)
    es = rng.integers(0, ns, e)
    ed = rng.integers(0, nd, e)
    got = kernel(src_emb=semb, edge_src=es, edge_dst=ed, num_dst=nd)
    exp = np.zeros((nd, 64), np.float32)
    np.add.at(exp, ed, semb[es])
    rel = np.abs(got - exp).max() / np.abs(exp).max()
    print("small-case rel err:", rel)



# revision 2
# speedup vs baseline: 2.2488x; 2.2488x over previous
"""GNN message passing (copy_u + segment_sum) on 8 Trainium2 cores.

Strategy (edge/data parallel, per the sharding hint):
  - Host: sort edges by dst; core c owns dst range [c*N/8, (c+1)*N/8).
  - Host: pad each dst's edge list to a per-class segment size m (classes
    4..128); a subtile of 128 edge slots holds floor(128/m) whole segments.
    Gather per-edge messages (fp16) per core, subtile-major so device DMAs
    are big contiguous runs per partition.
  - Device (per core): for each superbatch of 32 subtiles, run 8 col-tiled
    matmuls (N=256) against per-strip constant 0/1 segment matrices streamed
    as data -> PSUM [128,512] holds all segment sums; evacuate to fp16 SBUF
    (alternating Vector/Scalar engines) and batch-store.
  - Host: scatter-add the per-segment partial sums into the full output.
No per-bin one-hot build on DVE and no per-matmul 128-col weight reloads:
segment structure lives in tiny [128,32] stationaries shipped as data.
"""
import sys
sys.path.insert(0, "/opt/trn_rl_repo")
import numpy as np

import concourse.bass as bass
import concourse.bacc as bacc
import concourse.mybir as mybir
import concourse.tile as tile
from concourse.bass_utils import run_bass_kernel_spmd

NCORES = 8
SUB_PER_STRIP = 4          # subtiles per strip (one matmul, N=256)
STRIPS_PER_SB = 8          # strips per superbatch (one PSUM bank [128, 512])
SUB_PER_SB = SUB_PER_STRIP * STRIPS_PER_SB  # 32

CLASSES = np.array([4, 5, 6, 7, 8, 9, 10, 11, 12, 13, 14, 15, 16, 18, 21, 25,
                    32, 42, 64, 128])

_kernel_cache = {}


def _build_kernel(B):
    """Device program, uniform over cores; B superbatches."""
    f16 = mybir.dt.float16
    f32 = mybir.dt.float32
    nc = bacc.Bacc("TRN2", target_bir_lowering=False, debug=False,
                   num_devices=NCORES)
    msg = nc.declare_dram_parameter("msg", [128, B * 2048], f16, isOutput=False)
    rst = nc.declare_dram_parameter("rst", [128, B * 256], f16, isOutput=False)
    outp = nc.declare_dram_parameter("outp", [128, B * 512], f16, isOutput=True)

    G = 5  # superbatches per DMA group
    n_groups = (B + G - 1) // G

    with tile.TileContext(nc) as tc:
        with tc.tile_pool(name="msgs", bufs=3) as mpool, \
             tc.tile_pool(name="rsts", bufs=3) as rpool, \
             tc.tile_pool(name="acc", bufs=8, space="PSUM") as ppool, \
             tc.tile_pool(name="ost", bufs=3) as opool:
            for g in range(n_groups):
                g0 = g * G
                gs = min(G, B - g0)
                mt = mpool.tile([128, gs * 2048], f16, tag="mt")
                nc.sync.dma_start(out=mt[:], in_=msg[:, g0 * 2048:(g0 + gs) * 2048])
                rt = rpool.tile([128, gs * 256], f16, tag="rt")
                nc.sync.dma_start(out=rt[:], in_=rst[:, g0 * 256:(g0 + gs) * 256])
                ot = opool.tile([128, gs * 512], f16, tag="ot")
                for lsb in range(gs):
                    ps = ppool.tile([128, 512], f32)
                    for s in range(STRIPS_PER_SB):
                        j, h = s // 2, s % 2
                        nc.tensor.matmul(
                            ps[32 * j:32 * (j + 1), h * 256:(h + 1) * 256],
                            rt[:, lsb * 256 + s * 32:lsb * 256 + (s + 1) * 32],
                            mt[:, lsb * 2048 + s * 256:lsb * 2048 + (s + 1) * 256],
                            start=True, stop=True, tile_position=(0, 32 * j))
                    dst = ot[:, lsb * 512:(lsb + 1) * 512]
                    if (g0 + lsb) % 2 == 0:
                        nc.vector.tensor_copy(out=dst, in_=ps[:])
                    else:
                        nc.scalar.copy(out=dst, in_=ps[:])
                nc.sync.dma_start(out=outp[:, g0 * 512:(g0 + gs) * 512], in_=ot[:])
    nc.compile()
    return nc


def _pack_core(d_local, s_local):
    """Pack one core's dst-sorted edges into classed segment subtiles.

    Returns:
      src_of_slot [n_sub, 128] int64 (-1 = pad slot)
      m_of_subtile [n_sub] int64 (segment size class)
      row_of_seg [n_sub, 32] int64 (-1 = unused seg), local dst row per segment
    Subtiles are class-contiguous; each class is padded to a multiple of
    SUB_PER_STRIP subtiles so strips are class-pure.
    """
    n = len(d_local)
    if n == 0:
        return (np.full((0, 128), -1, np.int64), np.zeros(0, np.int64),
                np.full((0, 32), -1, np.int64))
    newdst = np.concatenate(([True], d_local[1:] != d_local[:-1]))
    first_pos = np.flatnonzero(newdst)
    first_idx = np.repeat(first_pos, np.diff(np.concatenate((first_pos, [n]))))
    rank = np.arange(n) - first_idx
    chunk = rank // 128                      # dst with >128 edges -> chunks
    r_in_entry = rank - 128 * chunk
    entry_break = np.concatenate(
        ([True], (d_local[1:] != d_local[:-1]) | (chunk[1:] != chunk[:-1])))
    entry_id = np.cumsum(entry_break) - 1
    n_entries = int(entry_id[-1]) + 1
    entry_first = np.flatnonzero(entry_break)
    entry_deg = np.diff(np.concatenate((entry_first, [n])))
    entry_dst = d_local[entry_first]

    ci = np.searchsorted(CLASSES, entry_deg)
    m_of_entry = CLASSES[ci]
    order = np.argsort(ci, kind="stable")
    cls_counts = np.bincount(ci, minlength=len(CLASSES))
    cls_start = np.concatenate(([0], np.cumsum(cls_counts)[:-1]))
    pos_in_class = np.empty(n_entries, dtype=np.int64)
    pos_in_class[order] = np.arange(n_entries) - cls_start[ci[order]]

    segs_of_class = 128 // CLASSES
    segs_of_entry = segs_of_class[ci]
    sub_in_class = pos_in_class // segs_of_entry
    g_of_entry = pos_in_class % segs_of_entry
    n_sub_class = -(-cls_counts // segs_of_class) * (cls_counts > 0)
    n_sub_class_pad = -(-n_sub_class // SUB_PER_STRIP) * SUB_PER_STRIP
    sub_base = np.concatenate(([0], np.cumsum(n_sub_class_pad)[:-1]))
    subtile_of_entry = sub_base[ci] + sub_in_class
    n_subtiles = int(n_sub_class_pad.sum())

    src_of_slot = np.full((n_subtiles, 128), -1, dtype=np.int64)
    row_of_seg = np.full((n_subtiles, 32), -1, dtype=np.int64)
    m_of_subtile = np.full(n_subtiles, 128, dtype=np.int64)
    for k in range(len(CLASSES)):
        if n_sub_class_pad[k]:
            m_of_subtile[sub_base[k]:sub_base[k] + n_sub_class_pad[k]] = CLASSES[k]
    slot_p = g_of_entry[entry_id] * m_of_entry[entry_id] + r_in_entry
    src_of_slot[subtile_of_entry[entry_id], slot_p] = s_local
    row_of_seg[subtile_of_entry, g_of_entry] = entry_dst
    return src_of_slot, m_of_subtile, row_of_seg


def kernel(src_emb, edge_src, edge_dst, num_dst):
    src_emb = np.asarray(src_emb, dtype=np.float32)
    edge_src = np.asarray(edge_src).astype(np.int64)
    edge_dst = np.asarray(edge_dst).astype(np.int64)
    n_dst = int(num_dst)
    n_src, d = src_emb.shape
    assert d == 64

    src16 = src_emb.astype(np.float16)

    order = np.argsort(edge_dst, kind="stable")
    ds = edge_dst[order]
    ss = edge_src[order]
    per = (n_dst + NCORES - 1) // NCORES
    cuts = np.searchsorted(ds, np.arange(1, NCORES) * per)
    d_parts = np.split(ds, cuts)
    s_parts = np.split(ss, cuts)

    cores = [_pack_core(d_parts[c] - c * per, s_parts[c]) for c in range(NCORES)]
    B = max(-(-cr[0].shape[0] // SUB_PER_SB) for cr in cores)
    B = max(B, 1)
    n_sub_pad = B * SUB_PER_SB

    # R pattern per class, precomputed [len(CLASSES), 128, 32]
    jj = np.arange(128)[:, None]
    gg = np.arange(32)[None, :]
    r_of_class = np.zeros((len(CLASSES), 128, 32), dtype=np.float16)
    for k, m in enumerate(CLASSES):
        segs = 128 // m
        r_of_class[k] = ((jj // m == gg) & (gg < segs) & (jj < m * segs))
    class_idx = {int(m): k for k, m in enumerate(CLASSES)}

    in_maps = []
    rowmaps = []
    for c in range(NCORES):
        src_of_slot, m_of_subtile, row_of_seg = cores[c]
        n_sub = src_of_slot.shape[0]
        if n_sub < n_sub_pad:
            src_of_slot = np.concatenate(
                [src_of_slot, np.full((n_sub_pad - n_sub, 128), -1, np.int64)])
            m_of_subtile = np.concatenate(
                [m_of_subtile, np.full(n_sub_pad - n_sub, 128, np.int64)])
            row_of_seg = np.concatenate(
                [row_of_seg, np.full((n_sub_pad - n_sub, 32), -1, np.int64)])

        # messages [128 slot, n_sub, 64] fp16, zero at pad slots
        msg3 = np.zeros((128, n_sub_pad, 64), dtype=np.float16)
        valid = src_of_slot >= 0                      # [n_sub, 128]
        sub_i, slot_i = np.nonzero(valid)
        msg3[slot_i, sub_i] = src16[src_of_slot[sub_i, slot_i]]
        msg_np = msg3.reshape(128, n_sub_pad * 64)

        # stationaries [128, n_strips*32] fp16 (strips are class-pure)
        m_of_strip = m_of_subtile[::SUB_PER_STRIP]
        ks = np.array([class_idx[int(m)] for m in m_of_strip])
        rst_np = np.ascontiguousarray(
            r_of_class[ks].transpose(1, 0, 2).reshape(128, -1))

        # rowmap aligned with out[128, B*8 col-chunks, 64]:
        # subtile t_glob = sb*32 + s*4 + t -> out[32*(s//2)+g,
        #   chunk = sb*8 + (s%2)*4 + t]
        rowmap = np.full((128, B * 8), n_dst, dtype=np.int64)
        t_glob = np.arange(n_sub_pad)
        sb, rem = t_glob // SUB_PER_SB, t_glob % SUB_PER_SB
        s, t = rem // SUB_PER_STRIP, rem % SUB_PER_STRIP
        chunk_of_sub = sb * 8 + (s % 2) * 4 + t
        pbase_of_sub = 32 * (s // 2)
        sub_i, g_i = np.nonzero(row_of_seg >= 0)
        glob_rows = row_of_seg[sub_i, g_i] + c * per
        rowmap[pbase_of_sub[sub_i] + g_i, chunk_of_sub[sub_i]] = glob_rows
        rowmaps.append(rowmap)
        in_maps.append({"msg": msg_np, "rst": rst_np})

    if B not in _kernel_cache:
        _kernel_cache[B] = _build_kernel(B)
    nc = _kernel_cache[B]
    res = run_bass_kernel_spmd(nc, in_maps, core_ids=list(range(NCORES)))

    full = np.zeros((n_dst + 1, 64), dtype=np.float32)
    for c in range(NCORES):
        blocks = res.results[c]["outp"].reshape(128, B * 8, 64).astype(np.float32)
        np.add.at(full, rowmaps[c].ravel(), blocks.reshape(-1, 64))
    return full[:n_dst]


if __name__ == "__main__":
    rng = np.random.default_rng(1)
    ns, nd, e = 1000, 1000, 5000
    semb = rng.standard_normal((ns, 64), dtype=np.float32)
    es = rng.integers(0, ns, e)
    ed = rng.integers(0, nd, e)
    got = kernel(src_emb=semb, edge_src=es, edge_dst=ed, num_dst=nd)
    exp = np.zeros((nd, 64), np.float32)
    np.add.at(exp, ed, semb[es])
    rel = np.abs(got - exp).max() / np.abs(exp).max()
    print("small-case rel err:", rel)
